# revision 60
# baseline (speedup 1.0000x reference)
import numpy as np
import ml_dtypes

import concourse.bass as bass
import concourse.tile as tile
from concourse import mybir
from concourse.bass_utils import run_bass_kernel_spmd
from concourse.vector_clock import ScopedClock

F32 = mybir.dt.float32
BF = mybir.dt.bfloat16
AF = mybir.ActivationFunctionType
OP = mybir.AluOpType

B, S, D = 8, 1024, 1024
H, HD, L, FF, NOUT, WIN = 8, 128, 4, 4096, 512, 32
ND = D // 128
NT = S // 128
NF = FF // 128
NFC = 4
FCT = NF // NFC
SC = 512
NSC = S // SC
SCALE = 1.0 / float(np.sqrt(HD))
EPS = 1e-5
N_CORES = 8


class SafeTileContext(tile.TileContext):

    def _drain_and_barrier(self, tick_clock, wait_clock):
        gclock = tick_clock.global_clock
        for proc in range(len(gclock)):
            tick = gclock[proc]
            if tick > 0:
                partial = ScopedClock()
                partial.require_at_least(None, proc, tick)
                nop = self.nc.sync.nop(nofuse=True)
                wait_clock.add_sem_waits(nop.ins, partial)
        self.nc.sync.drain()
        self.nc.all_engine_barrier()
        popped = self.nc._tile_sem_poison_stack.pop()
        assert popped is self._sem_poison
        self.nc.clear_and_free_semaphores(list(self.sems.allocated().values()))
        self.nc.all_engine_barrier()


def _split_multi_waits(nc):
    n = 0
    for f in nc.m.functions:
        for bb in f.blocks:
            insts = bb.instructions
            out = []
            for inst in insts:
                si = inst.sync_info
                waits = list(si.on_wait) if si is not None else []
                if len(waits) > 1:
                    for w in waits[:-1]:
                        n += 1
                        nop = mybir.InstNoOp(
                            name=f"{inst.name}-wsplit{n}",
                            engine=inst.engine,
                            ins=[], outs=[],
                            sync_info=mybir.SyncInfo(on_wait=[w], on_update=[]),
                        )
                        out.append(nop)
                    inst.sync_info = mybir.SyncInfo(
                        on_wait=[waits[-1]], on_update=list(si.on_update)
                    )
                out.append(inst)
            insts[:] = out
    return n


def build_program():
    nc = bass.Bass()

    xt = nc.dram_tensor("xt", [D, S], BF, kind="ExternalInput")
    ident = nc.dram_tensor("ident", [128, 128], BF, kind="ExternalInput")
    masks = nc.dram_tensor("masks", [128, 2, 256], F32, kind="ExternalInput")
    wqt = nc.dram_tensor("wqt", [L, D, D], BF, kind="ExternalInput")
    wkt = nc.dram_tensor("wkt", [L, D, D], BF, kind="ExternalInput")
    wvt = nc.dram_tensor("wvt", [L, D, D], BF, kind="ExternalInput")
    wot = nc.dram_tensor("wot", [L, D, D], BF, kind="ExternalInput")
    w1t = nc.dram_tensor("w1t", [L, D, FF], BF, kind="ExternalInput")
    w2t = nc.dram_tensor("w2t", [L, FF, D], BF, kind="ExternalInput")
    wht = nc.dram_tensor("wht", [D, NOUT], BF, kind="ExternalInput")
    bqt = nc.dram_tensor("bqt", [L, 128, ND], F32, kind="ExternalInput")
    bkt = nc.dram_tensor("bkt", [L, 128, ND], F32, kind="ExternalInput")
    bot = nc.dram_tensor("bot", [L, 128, ND], F32, kind="ExternalInput")
    b2t = nc.dram_tensor("b2t", [L, 128, ND], F32, kind="ExternalInput")
    b1t = nc.dram_tensor("b1t", [L, 128, NF], F32, kind="ExternalInput")
    bvt = nc.dram_tensor("bvt", [L, 128, H], F32, kind="ExternalInput")
    bhv = nc.dram_tensor("bhv", [1, NOUT], F32, kind="ExternalInput")
    g1t = nc.dram_tensor("g1t", [L, 128, ND], F32, kind="ExternalInput")
    h1t = nc.dram_tensor("h1t", [L, 128, ND], F32, kind="ExternalInput")
    g2t = nc.dram_tensor("g2t", [L, 128, ND], F32, kind="ExternalInput")
    h2t = nc.dram_tensor("h2t", [L, 128, ND], F32, kind="ExternalInput")
    onesc = nc.dram_tensor("onesc", [128, 1], BF, kind="ExternalInput")
    onesr = nc.dram_tensor("onesr", [1, 128], BF, kind="ExternalInput")
    y = nc.dram_tensor("out", [S, NOUT], F32, kind="ExternalOutput")

    with SafeTileContext(nc) as tc:
        from contextlib import ExitStack

        with ExitStack() as ctx:
            p_mm = ctx.enter_context(tc.tile_pool(name="p_mm", bufs=2, space="PSUM"))
            wpool = ctx.enter_context(tc.tile_pool(name="wpool", bufs=34))
            big = ctx.enter_context(tc.tile_pool(name="big", bufs=1))
            roll = ctx.enter_context(tc.tile_pool(name="roll", bufs=3))
            sqp = ctx.enter_context(tc.tile_pool(name="sqp", bufs=10))
            prp = ctx.enter_context(tc.tile_pool(name="prp", bufs=5))
            smp = ctx.enter_context(tc.tile_pool(name="smp", bufs=4))
            cst = ctx.enter_context(tc.tile_pool(name="cst", bufs=1))
            bias = ctx.enter_context(tc.tile_pool(name="bias", bufs=2))
            lnp = ctx.enter_context(tc.tile_pool(name="lnp", bufs=1))

            idt = cst.tile([128, 128], BF, tag="idt")
            nc.sync.dma_start(out=idt, in_=ident[:, :])
            msk = cst.tile([128, 2, 256], F32, tag="msk")
            nc.sync.dma_start(out=msk, in_=masks[:, :, :])
            bhb = cst.tile([128, NOUT], F32, tag="bhb")
            nc.sync.dma_start(out=bhb, in_=bhv[0:1, :].to_broadcast([128, NOUT]))
            ones_col = cst.tile([128, 1], BF, tag="onc")
            nc.sync.dma_start(out=ones_col, in_=onesc[:, :])
            ones_row = cst.tile([1, 128], BF, tag="onr")
            nc.sync.dma_start(out=ones_row, in_=onesr[:, :])
            epst = cst.tile([1, 1], F32, tag="eps")
            nc.vector.memset(epst, EPS)

            xT = big.tile([128, ND, S], BF, tag="xT")
            for di in range(ND):
                nc.sync.dma_start(
                    out=xT[:, di, :], in_=xt[di * 128:(di + 1) * 128, :]
                )

            def mk_sq(src, c, di):
                sq = sqp.tile([128, SC], BF, tag="sq")
                nc.scalar.activation(
                    out=sq, in_=src[:, di, c * SC:(c + 1) * SC], func=AF.Square
                )
                return sq

            def mk_pair(src, c, t, sqs, pxs, pss):
                sl = slice(c * SC, (c + 1) * SC)
                px = prp.tile([128, SC], BF, tag="px")
                nc.vector.tensor_add(px, src[:, t - 1, sl], src[:, t, sl])
                ps = prp.tile([128, SC], BF, tag="ps")
                nc.vector.tensor_add(ps, sqs[(c, t - 1)], sqs[(c, t)])
                pxs[(c, t // 2)] = px
                pss[(c, t // 2)] = ps

            def ln_stats(src, st1, st2, c, pxs, pss):
                for di in range(ND // 2):
                    nc.tensor.matmul(
                        st1[0:1, c, :], lhsT=ones_col, rhs=pxs[(c, di)],
                        start=(di == 0), stop=(di == ND // 2 - 1),
                    )
                for di in range(ND // 2):
                    nc.tensor.matmul(
                        st2[0:1, c, :], lhsT=ones_col, rhs=pss[(c, di)],
                        start=(di == 0), stop=(di == ND // 2 - 1),
                    )

            def ln_rows(st1, st2, c, ab, bb):
                sl = slice(c * SC, (c + 1) * SC)
                mean = lnp.tile([1, S], F32, tag="mean")
                m2 = lnp.tile([1, S], F32, tag="tmp")
                alpha = lnp.tile([1, S], F32, tag="alpha")
                beta = lnp.tile([1, S], F32, tag="tmp2")
                nc.vector.tensor_scalar(
                    out=mean[0:1, sl], in0=st1[0:1, c, :],
                    scalar1=1.0 / D, scalar2=None, op0=OP.mult,
                )
                nc.vector.tensor_mul(m2[0:1, sl], mean[0:1, sl], mean[0:1, sl])
                nc.vector.scalar_tensor_tensor(
                    out=alpha[0:1, sl], in0=st2[0:1, c, :],
                    scalar=1.0 / D, in1=m2[0:1, sl],
                    op0=OP.mult, op1=OP.subtract,
                )
                nc.scalar.activation(
                    out=alpha[0:1, sl], in_=alpha[0:1, sl],
                    func=AF.Sqrt, bias=epst[0:1, 0:1],
                )
                nc.vector.reciprocal(alpha[0:1, sl], alpha[0:1, sl])
                nc.vector.scalar_tensor_tensor(
                    out=beta[0:1, sl], in0=mean[0:1, sl], scalar=-1.0,
                    in1=alpha[0:1, sl], op0=OP.mult, op1=OP.mult,
                )
                nc.vector.tensor_copy(out=ab[0:1, sl], in_=alpha[0:1, sl])
                nc.vector.tensor_copy(out=bb[0:1, sl], in_=beta[0:1, sl])

            def ln_bcast(c, ab, bb, abt, bbt, p_bc):
                sl = slice(c * SC, (c + 1) * SC)
                bc1 = p_bc.tile([128, SC], F32, tag="mm")
                nc.tensor.matmul(
                    bc1, lhsT=ones_row, rhs=ab[0:1, sl], start=True, stop=True,
                )
                nc.scalar.activation(out=abt[:, c, :], in_=bc1, func=AF.Identity)
                bc2 = p_bc.tile([128, SC], F32, tag="mm")
                nc.tensor.matmul(
                    bc2, lhsT=ones_row, rhs=bb[0:1, sl], start=True, stop=True,
                )
                nc.scalar.activation(out=bbt[:, c, :], in_=bc2, func=AF.Identity)

            def ln_apply(src, dst, c, abt, bbt, g, b):
                sl = slice(c * SC, (c + 1) * SC)
                for di in range(ND):
                    dslc = dst[:, di, sl]
                    nc.vector.tensor_mul(dslc, src[:, di, sl], abt[:, c, :])
                    nc.vector.tensor_add(dslc, dslc, bbt[:, c, :])
                    nc.scalar.activation(
                        out=dslc, in_=dslc, func=AF.Identity,
                        scale=g[:, di:di + 1], bias=b[:, di:di + 1],
                    )

            abt1 = lnp.tile([128, NSC, SC], BF, tag="abt")
            bbt1 = lnp.tile([128, NSC, SC], BF, tag="bbt")

            pending = None

            for l in range(L):
                bq = bias.tile([128, ND], F32, tag="bq")
                nc.sync.dma_start(out=bq, in_=bqt[l])
                bk = bias.tile([128, ND], F32, tag="bk")
                nc.sync.dma_start(out=bk, in_=bkt[l])
                bo = bias.tile([128, ND], F32, tag="bo")
                nc.sync.dma_start(out=bo, in_=bot[l])
                b2 = bias.tile([128, ND], F32, tag="b2")
                nc.sync.dma_start(out=b2, in_=b2t[l])
                b1 = bias.tile([128, NF], F32, tag="b1")
                nc.sync.dma_start(out=b1, in_=b1t[l])
                bvh = bias.tile([128, H], F32, tag="bvh")
                nc.sync.dma_start(out=bvh, in_=bvt[l])
                g1 = bias.tile([128, ND], F32, tag="g1")
                nc.sync.dma_start(out=g1, in_=g1t[l])
                h1 = bias.tile([128, ND], F32, tag="h1")
                nc.sync.dma_start(out=h1, in_=h1t[l])
                g2 = bias.tile([128, ND], F32, tag="g2")
                nc.sync.dma_start(out=g2, in_=g2t[l])
                h2 = bias.tile([128, ND], F32, tag="h2")
                nc.sync.dma_start(out=h2, in_=h2t[l])

                vT = big.tile([128, NT, D], BF, tag="big1")
                p_v = tc.tile_pool(name="p_v", bufs=6, space="PSUM")
                p_v_pool = p_v.__enter__()
                for dc in range(NSC):
                    wv = []
                    for di in range(ND):
                        wt = wpool.tile([128, SC], BF, tag="w")
                        nc.sync.dma_start(
                            out=wt,
                            in_=wvt[l, di * 128:(di + 1) * 128, dc * SC:(dc + 1) * SC],
                        )
                        wv.append(wt)
                    for st_ in range(NT):
                        ps = p_v_pool.tile([128, SC], F32, tag="mm")
                        for di in range(ND):
                            nc.tensor.matmul(
                                ps, lhsT=xT[:, di, st_ * 128:(st_ + 1) * 128],
                                rhs=wv[di],
                                start=(di == 0), stop=(di == ND - 1),
                            )
                        nc.scalar.activation(
                            out=vT[:, st_, dc * SC:(dc + 1) * SC], in_=ps,
                            func=AF.Identity,
                        )
                        if dc == 0 and pending is not None:
                            if st_ == 2:
                                pending[0]()
                            elif st_ == 3:
                                pending[1]()
                                pending = None
                p_v.__exit__(None, None, None)
                if pending is not None:
                    pending[0]()
                    pending[1]()
                    pending = None

                oT = big.tile([128, ND, S], BF, tag="big2")
                with tc.tile_pool(name="p_sc", bufs=2, space="PSUM") as p_sc, \
                     tc.tile_pool(name="p_tr", bufs=2, space="PSUM") as p_tr, \
                     tc.tile_pool(name="p_av", bufs=2, space="PSUM") as p_av:

                    def attn_chain(h, qb, kp, it):
                        scp = p_sc.tile([128, 256], F32, tag="sc")
                        nc.tensor.matmul(
                            scp, lhsT=qb[:, it * 128:(it + 1) * 128],
                            rhs=kp[:, it * 128:it * 128 + 256],
                            start=True, stop=True,
                        )
                        sm = smp.tile([128, 256], BF, tag="sm")
                        mi = 1 if it == 0 else 0
                        nc.vector.tensor_add(sm, scp, msk[:, mi, :])
                        rs = smp.tile([128, 1], F32, tag="rs")
                        nc.scalar.activation(
                            out=sm, in_=sm, func=AF.Exp, accum_out=rs
                        )
                        rc = smp.tile([128, 1], F32, tag="rc")
                        nc.vector.reciprocal(rc, rs)
                        nc.vector.tensor_scalar(
                            out=sm, in0=sm, scalar1=rc, scalar2=None,
                            op0=OP.mult,
                        )
                        jlo = 1 if it == 0 else 0
                        trp = p_tr.tile([128, 2, 128], BF, tag="tr")
                        for jj in range(jlo, 2):
                            nc.tensor.transpose(
                                trp[:, jj, :], sm[:, jj * 128:(jj + 1) * 128], idt
                            )
                        at = smp.tile([128, 2, 128], BF, tag="at")
                        nc.vector.tensor_copy(
                            out=at[:, jlo:2, :], in_=trp[:, jlo:2, :]
                        )
                        av = p_av.tile([128, 128], F32, tag="av")
                        for jj in range(jlo, 2):
                            jt = it - 1 + jj
                            nc.tensor.matmul(
                                av, lhsT=vT[:, jt, h * 128:(h + 1) * 128],
                                rhs=at[:, jj, :],
                                start=(jj == jlo), stop=(jj == 1),
                            )
                        nc.scalar.activation(
                            out=oT[:, h, it * 128:(it + 1) * 128], in_=av,
                            func=AF.Identity, bias=bvh[:, h:h + 1],
                        )

                    prev = None
                    for h in range(H):
                        wq = wpool.tile([128, ND, 128], BF, tag="w")
                        nc.sync.dma_start(
                            out=wq,
                            in_=wqt[l].rearrange("(a p) n -> p a n", p=128)[
                                :, :, h * 128:(h + 1) * 128],
                        )
                        wk = wpool.tile([128, ND, 128], BF, tag="w")
                        nc.sync.dma_start(
                            out=wk,
                            in_=wkt[l].rearrange("(a p) n -> p a n", p=128)[
                                :, :, h * 128:(h + 1) * 128],
                        )
                        qb = roll.tile([128, S], BF, tag="qh")
                        kp = roll.tile([128, 128 + S], BF, tag="kh")
                        nc.vector.memset(kp[:, 0:128], 0.0)

                        def qk_group(w, c, out, bias_, scale_):
                            ps = p_mm.tile([128, SC], F32, tag="mm")
                            for di in range(ND):
                                nc.tensor.matmul(
                                    ps, lhsT=w[:, di, :],
                                    rhs=xT[:, di, c * SC:(c + 1) * SC],
                                    start=(di == 0), stop=(di == ND - 1),
                                )
                            nc.scalar.activation(
                                out=out, in_=ps, func=AF.Identity,
                                bias=bias_, scale=scale_,
                            )

                        def pch(lo, hi):
                            if prev is not None:
                                for it in range(lo, hi):
                                    attn_chain(prev[0], prev[1], prev[2], it)

                        qk_group(wq, 0, qb[:, 0:SC], bq[:, h:h + 1], SCALE)
                        pch(2, 4)
                        qk_group(wk, 0, kp[:, 128:128 + SC], bk[:, h:h + 1], 1.0)
                        pch(4, 6)
                        qk_group(wq, 1, qb[:, SC:S], bq[:, h:h + 1], SCALE)
                        pch(6, 8)
                        qk_group(wk, 1, kp[:, 128 + SC:128 + S], bk[:, h:h + 1], 1.0)
                        if prev is not None:
                            attn_chain(prev[0], prev[1], prev[2], 0)
                            attn_chain(prev[0], prev[1], prev[2], 1)
                        prev = (h, qb, kp)
                    attn_chain(prev[0], prev[1], prev[2], 0)
                    attn_chain(prev[0], prev[1], prev[2], 1)
                    for it in range(2, NT):
                        attn_chain(prev[0], prev[1], prev[2], it)

                p_st1 = tc.tile_pool(name="p_st1", bufs=1, space="PSUM")
                p_st = p_st1.__enter__()
                stp = p_st.tile([64, NSC, SC], F32, tag="stp")
                st1 = stp[0:1]
                st2 = stp[32:33]
                p_oc = tc.tile_pool(name="p_oc", bufs=4, space="PSUM")
                p_o = p_oc.__enter__()
                wo_tiles = []
                for t in range(ND):
                    wo = wpool.tile([128, ND, 128], BF, tag="w")
                    nc.sync.dma_start(
                        out=wo,
                        in_=wot[l].rearrange("(a p) n -> p a n", p=128)[
                            :, :, t * 128:(t + 1) * 128],
                    )
                    wo_tiles.append(wo)
                ab1 = lnp.tile([1, S], BF, tag="ab")
                bb1 = lnp.tile([1, S], BF, tag="bb")
                sqs = {}
                pxs = {}
                pss = {}

                def oproj_tile(c, t):
                    ps = p_o.tile([128, SC], F32, tag="mm")
                    for di in range(ND):
                        nc.tensor.matmul(
                            ps, lhsT=wo_tiles[t][:, di, :],
                            rhs=oT[:, di, c * SC:(c + 1) * SC],
                            start=(di == 0), stop=(di == ND - 1),
                        )
                    ot = roll.tile([128, SC], BF, tag="dr")
                    nc.scalar.activation(
                        out=ot, in_=ps, func=AF.Identity, bias=bo[:, t:t + 1],
                    )
                    xslc = xT[:, t, c * SC:(c + 1) * SC]
                    nc.vector.tensor_add(xslc, xslc, ot)
                    sqs[(c, t)] = mk_sq(xT, c, t)
                    if t % 2 == 1:
                        mk_pair(xT, c, t, sqs, pxs, pss)

                for t in range(ND):
                    oproj_tile(0, t)
                for t in range(6):
                    oproj_tile(1, t)
                    if t == 1:
                        ln_stats(xT, st1, st2, 0, pxs, pss)
                        ln_rows(st1, st2, 0, ab1, bb1)
                ln_bcast(0, ab1, bb1, abt1, bbt1, p_o)
                for t in range(6, ND):
                    oproj_tile(1, t)
                ln_stats(xT, st1, st2, 1, pxs, pss)
                ln_apply(xT, xT, 0, abt1, bbt1, g1, h1)
                ln_rows(st1, st2, 1, ab1, bb1)
                p_oc.__exit__(None, None, None)
                p_st1.__exit__(None, None, None)

                ffr = big.tile([128, ND, S], BF, tag="big2")
                p_fc = tc.tile_pool(name="p_fc", bufs=4, space="PSUM")
                p_ffn = p_fc.__enter__()
                for fc in range(NFC):
                    hT = big.tile([128, FCT, S], BF, tag="big1")
                    w1 = []
                    for di in range(ND):
                        wt = wpool.tile([128, 1024], BF, tag="w")
                        nc.sync.dma_start(
                            out=wt,
                            in_=w1t[l, di * 128:(di + 1) * 128,
                                    fc * 1024:(fc + 1) * 1024],
                        )
                        w1.append(wt)
                    w2 = []
                    for ft in range(FCT):
                        wt = wpool.tile([128, 1024], BF, tag="w")
                        nc.sync.dma_start(
                            out=wt,
                            in_=w2t[l, (fc * FCT + ft) * 128:(fc * FCT + ft + 1) * 128, :],
                        )
                        w2.append(wt)
                    for c in range(NSC):
                        for ft in range(FCT):
                            ps = p_ffn.tile([128, SC], F32, tag="mm")
                            for di in range(ND):
                                nc.tensor.matmul(
                                    ps, lhsT=w1[di][:, ft * 128:(ft + 1) * 128],
                                    rhs=xT[:, di, c * SC:(c + 1) * SC],
                                    start=(di == 0), stop=(di == ND - 1),
                                )
                            nc.scalar.activation(
                                out=hT[:, ft, c * SC:(c + 1) * SC], in_=ps,
                                func=AF.Relu, bias=b1[:, fc * FCT + ft:fc * FCT + ft + 1],
                            )
                            if fc == 0 and c == 0:
                                if ft == 5:
                                    ln_bcast(1, ab1, bb1, abt1, bbt1, p_ffn)
                                elif ft == 6:
                                    ln_apply(xT, xT, 1, abt1, bbt1, g1, h1)

                    last = fc == NFC - 1
                    if last:
                        p_st2 = tc.tile_pool(name="p_st2", bufs=1, space="PSUM")
                        p_s2 = p_st2.__enter__()
                        stpb = p_s2.tile([64, NSC, SC], F32, tag="stp")
                        st1b = stpb[0:1]
                        st2b = stpb[32:33]
                        ab2 = lnp.tile([1, S], BF, tag="ab")
                        bb2 = lnp.tile([1, S], BF, tag="bb")
                        sqs2 = {}
                        pxs2 = {}
                        pss2 = {}

                    def ffn2_tile(c, t, last_):
                        ps = p_ffn.tile([128, SC], F32, tag="mm")
                        for ft in range(FCT):
                            nc.tensor.matmul(
                                ps, lhsT=w2[ft][:, t * 128:(t + 1) * 128],
                                rhs=hT[:, ft, c * SC:(c + 1) * SC],
                                start=(ft == 0), stop=(ft == FCT - 1),
                            )
                        fslc = ffr[:, t, c * SC:(c + 1) * SC]
                        ft2 = roll.tile([128, SC], BF, tag="dr")
                        if fc == 0:
                            nc.scalar.activation(
                                out=ft2, in_=ps, func=AF.Identity,
                                bias=b2[:, t:t + 1],
                            )
                            nc.vector.tensor_add(
                                fslc, xT[:, t, c * SC:(c + 1) * SC], ft2,
                            )
                        else:
                            nc.scalar.activation(out=ft2, in_=ps, func=AF.Identity)
                            nc.vector.tensor_add(fslc, fslc, ft2)
                        if last_:
                            sqs2[(c, t)] = mk_sq(ffr, c, t)
                            if t % 2 == 1:
                                mk_pair(ffr, c, t, sqs2, pxs2, pss2)

                    if not last:
                        for c in range(NSC):
                            for t in range(ND):
                                ffn2_tile(c, t, False)
                    else:
                        for t in range(ND):
                            ffn2_tile(0, t, True)
                        for t in range(6):
                            ffn2_tile(1, t, True)
                            if t == 1:
                                ln_stats(ffr, st1b, st2b, 0, pxs2, pss2)
                                ln_rows(st1b, st2b, 0, ab2, bb2)
                        ln_bcast(0, ab2, bb2, abt1, bbt1, p_ffn)
                        for t in range(6, ND):
                            ffn2_tile(1, t, True)
                        ln_stats(ffr, st1b, st2b, 1, pxs2, pss2)
                        ln_apply(ffr, xT, 0, abt1, bbt1, g2, h2)
                        ln_rows(st1b, st2b, 1, ab2, bb2)
                        p_st2.__exit__(None, None, None)

                        def mk_pending(ab_=ab2, bb_=bb2, ffr_=ffr, g_=g2, h_=h2):
                            def run_bc():
                                ln_bcast(1, ab_, bb_, abt1, bbt1, p_mm)
                            def run_apply():
                                ln_apply(ffr_, xT, 1, abt1, bbt1, g_, h_)
                            return run_bc, run_apply
                        pending = mk_pending()
                p_fc.__exit__(None, None, None)

            wh = []
            for di in range(ND):
                wt = wpool.tile([128, NOUT], BF, tag="w")
                nc.sync.dma_start(out=wt, in_=wht[di * 128:(di + 1) * 128, :])
                wh.append(wt)
            for st_ in range(NT):
                ps = p_mm.tile([128, NOUT], F32, tag="mm")
                for di in range(ND):
                    nc.tensor.matmul(
                        ps, lhsT=xT[:, di, st_ * 128:(st_ + 1) * 128],
                        rhs=wh[di],
                        start=(di == 0), stop=(di == ND - 1),
                    )
                ob = roll.tile([128, NOUT], F32, tag="ob")
                nc.vector.tensor_add(ob, ps, bhb)
                nc.sync.dma_start(out=y[st_ * 128:(st_ + 1) * 128, :], in_=ob)
                if pending is not None:
                    if st_ == 1:
                        pending[0]()
                    elif st_ == 3:
                        pending[1]()
                        pending = None

    _split_multi_waits(nc)
    return nc


def _host_prep(inputs):
    f32 = np.float32
    bf16 = ml_dtypes.bfloat16
    inp = {k: np.asarray(v, dtype=f32) if np.asarray(v).dtype != np.int32 else np.asarray(v)
           for k, v in inputs.items()}

    pos = np.arange(S, dtype=f32)[:, None]
    div = np.exp(np.arange(0, D, 2, dtype=f32) * (-np.log(10000.0) / D)).astype(f32)
    pe = np.zeros((S, D), f32)
    pe[:, 0::2] = np.sin(pos * div)
    pe[:, 1::2] = np.cos(pos * div)

    c = np.arange(256)[None, :]
    r = np.arange(128)[:, None]
    allowed_main = (c >= r + 97) & (c <= r + 128)
    allowed_first = allowed_main & (c >= 128)
    mask_main = np.where(allowed_main, 0.0, -1e9).astype(f32)
    mask_first = np.where(allowed_first, 0.0, -1e9).astype(f32)
    masks = np.ascontiguousarray(np.stack([mask_main, mask_first], axis=1))

    def colmajor(v):
        return np.ascontiguousarray(v.reshape(L, -1, 128).transpose(0, 2, 1))

    shared = {
        "ident": np.eye(128, dtype=bf16),
        "masks": masks,
        "wqt": np.ascontiguousarray(inp["Wq"].transpose(0, 2, 1)).astype(bf16),
        "wkt": np.ascontiguousarray(inp["Wk"].transpose(0, 2, 1)).astype(bf16),
        "wvt": np.ascontiguousarray(inp["Wv"].transpose(0, 2, 1)).astype(bf16),
        "wot": np.ascontiguousarray(inp["Wo"].transpose(0, 2, 1)).astype(bf16),
        "w1t": np.ascontiguousarray(inp["W1"].transpose(0, 2, 1)).astype(bf16),
        "w2t": np.ascontiguousarray(inp["W2"].transpose(0, 2, 1)).astype(bf16),
        "wht": np.ascontiguousarray(inp["Wh"].T).astype(bf16),
        "bqt": colmajor(inp["bq"]),
        "bkt": colmajor(inp["bk"]),
        "bot": colmajor(inp["bo"]),
        "b2t": colmajor(inp["b2"]),
        "b1t": colmajor(inp["b1"]),
        "bvt": colmajor(inp["bv"]),
        "bhv": np.ascontiguousarray(inp["bh"].reshape(1, NOUT)),
        "g1t": colmajor(inp["ln1_g"]),
        "h1t": colmajor(inp["ln1_b"]),
        "g2t": colmajor(inp["ln2_g"]),
        "h2t": colmajor(inp["ln2_b"]),
        "onesc": np.ones((128, 1), bf16),
        "onesr": np.ones((1, 128), bf16),
    }
    in_maps = []
    for b in range(N_CORES):
        m = dict(shared)
        m["xt"] = np.ascontiguousarray((inp["X"][b] + pe).T).astype(bf16)
        in_maps.append(m)
    return in_maps


_NC_CACHE = {}


def run(inputs, trace=False, **spmd_kwargs):
    if "nc" not in _NC_CACHE:
        _NC_CACHE["nc"] = build_program()
    nc = _NC_CACHE["nc"]
    in_maps = _host_prep(inputs)
    res = run_bass_kernel_spmd(
        nc, in_maps, list(range(N_CORES)), trace=trace, **spmd_kwargs
    )
    out = np.concatenate([res.results[i]["out"] for i in range(N_CORES)], axis=0)
    return out, res


def kernel(**inputs) -> np.ndarray:
    out, _ = run(inputs, trace=False)
    return out
